# revision 8
# baseline (speedup 1.0000x reference)
"""BertSelfAttention on 8 Trainium2 NeuronCores (Bass/Tile, SPMD) — v2.

Problem: B=2, S=2048, D=1024, H=16 heads, head_dim=64.
Sharding: core c handles batch b = c//4 and heads [4*(c%4), 4*(c%4)+4)
(data parallel on B x tensor parallel on heads). Scores stay core-local.

v2 design (vs v1 baseline at 262us):
  - ACT(exp) is the hard floor: 16.8M probs/core at 1 elem/lane/cycle
    @1.2GHz + ~280cyc/call => ~141us for 128 [128,1024] calls. The whole
    kernel is scheduled so the exp stream NEVER stalls:
      * pT staged in SBUF (deep ring) decouples ctx matmuls from exp.
      * ~10 kc of scores+exp prefetched while the V projection runs.
      * scores matmuls are pair-tiled: heads 2j/2j+1 run CONCURRENTLY in
        the PE via tile_position row groups (0,0)/(64,0) with K=64 (kT
        stored compact, no zero padding) -> PE well under ACT per kc.
  - bf16 everywhere on inputs/activations (halves DMA + SBUF; matmul
    rate on PE is column-driven, so bf16 == fp32r there). PSUM fp32.
  - out-DMAs alternate sync/gpsimd queues to kill the tail.

Math notes (exact transformations vs the reference):
  - bk dropped: scores[i,j] += q_i . bk is constant in j -> softmax invariant.
  - bv added host-side: softmax rows sum to 1 -> probs @ (1 x bv) = bv.
  - no max-subtraction: scores ~ N(0,1), exp range is tiny for fp32.
  - additive mask folded multiplicatively: exp(s+m) = exp(s)*exp(m), with
    exp(mask) baked into V columns and a 65th "ones" column per head whose
    ctx row is the softmax denominator.
"""

import numpy as np
from contextlib import ExitStack

B, S, D, H = 2, 2048, 1024, 16
HD = 64
N_CORES = 8
HPC = 4            # heads per core
CW = HPC * HD      # 256 output cols per core
KI = D // 128      # 8 contraction chunks
NSC = S // 128     # 16 k-chunks of 128
NPRE = 16          # kc iterations of (scores, exp) prefetched for (j0, qh0)

_NC_CACHE = []


def _build_nc():
    import concourse.bacc as bacc
    import concourse.mybir as mybir
    import concourse.tile as tile
    from concourse import masks

    F32 = mybir.dt.float32
    BF16 = mybir.dt.bfloat16
    AF = mybir.ActivationFunctionType

    nc = bacc.Bacc("TRN2", target_bir_lowering=False, debug=False)

    hsT_d = nc.dram_tensor("hsT", [D, S], BF16, kind="ExternalInput")
    wq_d = nc.dram_tensor("wq", [D, CW], BF16, kind="ExternalInput")
    wk_d = nc.dram_tensor("wk", [D, CW], BF16, kind="ExternalInput")
    wv_d = nc.dram_tensor("wv", [D, CW], BF16, kind="ExternalInput")
    bq_d = nc.dram_tensor("bq", [CW], F32, kind="ExternalInput")
    mask_d = nc.dram_tensor("mask", [S], F32, kind="ExternalInput")
    out_d = nc.dram_tensor("out", [S, CW], F32, kind="ExternalOutput")

    hsT_r = hsT_d.rearrange("(ki p) s -> p ki s", p=128)
    wq_r = wq_d.rearrange("(ki p) n -> p ki n", p=128)
    wk_r = wk_d.rearrange("(ki p) n -> p ki n", p=128)
    wv_r = wv_d.rearrange("(ki p) n -> p ki n", p=128)
    bq_r = bq_d.rearrange("(j p) -> p j", p=128)
    mask_r = mask_d.rearrange("(sc p) -> p sc", p=128)

    with tile.TileContext(nc) as tc, ExitStack() as ctx:
        const = ctx.enter_context(tc.tile_pool(name="const", bufs=1))
        load = ctx.enter_context(tc.tile_pool(name="load", bufs=1))
        acts = ctx.enter_context(tc.tile_pool(name="acts", bufs=1))
        ptp = ctx.enter_context(tc.tile_pool(name="ptp", bufs=2 * NPRE + 6))
        ctsp = ctx.enter_context(tc.tile_pool(name="ctsp", bufs=4))
        outp = ctx.enter_context(tc.tile_pool(name="outp", bufs=8))
        # PSUM (8 banks): sp 2x[128,1024] = 4 banks; acc 2x[65,1024] = 4
        # banks (transposes reuse the acc ring between qh phases).
        sp = ctx.enter_context(tc.tile_pool(name="sp", bufs=2, space="PSUM"))
        accp = ctx.enter_context(tc.tile_pool(name="accp", bufs=2, space="PSUM"))

        # ---- constants ----
        ident = const.tile([128, 128], BF16)
        masks.make_identity(nc, ident[:])
        bq_t = const.tile([128, 2], F32)
        nc.sync.dma_start(bq_t[:], bq_r)
        maskr = const.tile([128, 16], F32)
        nc.sync.dma_start(maskr[:], mask_r)
        emt = const.tile([128, 16], F32)
        nc.scalar.activation(emt[:], maskr[:], AF.Exp)

        # ---- loads ----
        hsT_t = load.tile([128, KI, S], BF16)
        wq_t = load.tile([128, KI, CW], BF16)
        wk_t = load.tile([128, KI, CW], BF16)
        wv_t = load.tile([128, KI, CW], BF16)
        for ki in range(KI):
            nc.gpsimd.dma_start(wq_t[:, ki, :], wq_r[:, ki, :])
        for ki in range(KI):
            nc.gpsimd.dma_start(wk_t[:, ki, :], wk_r[:, ki, :])
        # hsT per (n, ki) so qk(j0, n) can start as chunks land
        for n in range(4):
            for ki in range(KI):
                nc.sync.dma_start(hsT_t[:, ki, 512 * n:512 * (n + 1)],
                                  hsT_r[:, ki, 512 * n:512 * (n + 1)])
        for ki in range(KI):
            nc.gpsimd.dma_start(wv_t[:, ki, :], wv_r[:, ki, :])

        # ---- persistent activations ----
        qT_t = acts.tile([128, 2, S], BF16)
        kz = acts.tile([128, 2, S], BF16)
        v_ext = acts.tile([128, NSC, HPC * (HD + 1)], BF16)

        def emit_qk_n(j, n):
                ps = sp.tile([128, 1024], F32, tag="sp", name=f"qk{j}_{n}")
                for ki in range(KI):
                    nc.tensor.matmul(
                        ps[:, 0:512], wq_t[:, ki, 128 * j:128 * (j + 1)],
                        hsT_t[:, ki, 512 * n:512 * (n + 1)],
                        start=(ki == 0), stop=(ki == KI - 1))
                for ki in range(KI):
                    nc.tensor.matmul(
                        ps[:, 512:1024], wk_t[:, ki, 128 * j:128 * (j + 1)],
                        hsT_t[:, ki, 512 * n:512 * (n + 1)],
                        start=(ki == 0), stop=(ki == KI - 1))
                nc.vector.tensor_scalar_add(qT_t[:, j, 512 * n:512 * (n + 1)],
                                            ps[:, 0:512], bq_t[:, j:j + 1])
                nc.vector.tensor_copy(kz[:, j, 512 * n:512 * (n + 1)],
                                      ps[:, 512:1024])

        def emit_v_chunk(sc):
                ps = sp.tile([128, 1024], F32, tag="sp", name=f"v{sc}")
                for ki in range(KI):
                    nc.tensor.matmul(
                        ps[:, 0:CW], hsT_t[:, ki, 128 * sc:128 * (sc + 1)],
                        wv_t[:, ki, :], start=(ki == 0), stop=(ki == KI - 1))
                for h in range(HPC):
                    nc.vector.tensor_scalar_mul(
                        v_ext[:, sc, 65 * h:65 * h + 64],
                        ps[:, 64 * h:64 * (h + 1)], emt[:, sc:sc + 1])
                    nc.vector.tensor_copy(
                        v_ext[:, sc, 65 * h + 64:65 * h + 65], emt[:, sc:sc + 1])

        def emit_scores_exp(j, qh, kc):
            """Pair-tiled scores for heads (2j, 2j+1) + exp -> (pA, pB)."""
            spA = sp.tile([128, 1024], F32, tag="sp", name=f"sA{j}_{qh}_{kc}")
            spB = sp.tile([128, 1024], F32, tag="sp", name=f"sB{j}_{qh}_{kc}")
            q0 = 1024 * qh
            for half in range(2):
                qs = slice(q0 + 512 * half, q0 + 512 * (half + 1))
                os_ = slice(512 * half, 512 * (half + 1))
                nc.tensor.matmul(
                    spA[:, os_], kz[0:64, j, 128 * kc:128 * (kc + 1)],
                    qT_t[0:64, j, qs], start=True, stop=True,
                    tile_position=(0, 0))
                nc.tensor.matmul(
                    spB[:, os_], kz[64:128, j, 128 * kc:128 * (kc + 1)],
                    qT_t[64:128, j, qs], start=True, stop=True,
                    tile_position=(64, 0))
            pA = ptp.tile([128, 1024], BF16, tag="pt", name=f"pA{j}_{qh}_{kc}")
            pB = ptp.tile([128, 1024], BF16, tag="pt", name=f"pB{j}_{qh}_{kc}")
            nc.scalar.activation(pA[:], spA[:], AF.Exp, scale=0.125)
            nc.scalar.activation(pB[:], spB[:], AF.Exp, scale=0.125)
            return pA, pB

        def emit_attention(j, qh, pre):
            accA = accp.tile([65, 1024], F32, tag="acc", name=f"aA{j}_{qh}")
            accB = accp.tile([65, 1024], F32, tag="acc", name=f"aB{j}_{qh}")
            for kc in range(NSC):
                if (j, qh, kc) in pre:
                    pA, pB = pre.pop((j, qh, kc))
                else:
                    pA, pB = emit_scores_exp(j, qh, kc)
                for acc, p, hh in ((accA, pA, 0), (accB, pB, 1)):
                    h = 2 * j + hh
                    for half in range(2):
                        os_ = slice(512 * half, 512 * (half + 1))
                        nc.tensor.matmul(
                            acc[0:65, os_], v_ext[:, kc, 65 * h:65 * (h + 1)],
                            p[:, os_], start=(kc == 0), stop=(kc == NSC - 1))
            # epilogue: transpose, normalize, store
            for hh, acc in ((0, accA), (1, accB)):
                h = 2 * j + hh
                cts = ctsp.tile([65, 1024], BF16, tag="cts")
                nc.vector.tensor_copy(cts[:], acc[0:65, :])
                for sq in range(8):
                    tp = accp.tile([128, 65], BF16, tag="acc",
                                   name=f"tp{j}_{qh}_{hh}_{sq}")
                    nc.tensor.transpose(tp[:, 0:65],
                                        cts[0:65, 128 * sq:128 * (sq + 1)],
                                        ident[0:65, 0:65])
                    rec = outp.tile([128, 1], F32, tag="rec")
                    nc.vector.reciprocal(rec[:], tp[:, 64:65])
                    ot = outp.tile([128, HD], F32, tag="ot")
                    nc.vector.tensor_scalar_mul(ot[:], tp[:, 0:HD], rec[:, 0:1])
                    row = 1024 * qh + 128 * sq
                    eng = nc.sync if sq % 2 == 0 else nc.gpsimd
                    eng.dma_start(
                        out_d[row:row + 128, HD * h:HD * (h + 1)], ot[:])

        # ---- schedule ----
        # qk(j0) gates the exp stream; v + qk(j1) are "aux" PE work threaded
        # through the prefetch loop so the sp-ring (strict FIFO) never puts
        # them between score tiles the exp stream is waiting on.
        for n in range(4):
            emit_qk_n(0, n)
        aux = [(lambda sc=sc: emit_v_chunk(sc)) for sc in range(NSC)]
        aux += [(lambda n=n: emit_qk_n(1, n)) for n in range(4)]
        pre = {}
        ai = 0
        for kc in range(NPRE):
            pre[(0, 0, kc)] = emit_scores_exp(0, 0, kc)
            take = 2 if kc < len(aux) - NPRE else 1
            for _ in range(take):
                if ai < len(aux):
                    aux[ai]()
                    ai += 1
        while ai < len(aux):
            aux[ai]()
            ai += 1
        emit_attention(0, 0, pre)
        emit_attention(0, 1, pre)
        emit_attention(1, 0, pre)
        emit_attention(1, 1, pre)

    nc.finalize()
    return nc


def _get_nc():
    if not _NC_CACHE:
        _NC_CACHE.append(_build_nc())
    return _NC_CACHE[0]


def _shard_inputs(hidden_states, attention_mask, Wq, bq, Wk, Wv):
    import ml_dtypes
    bf16 = ml_dtypes.bfloat16
    hsT = [np.ascontiguousarray(hidden_states[b].T).astype(bf16)
           for b in range(B)]
    wq_b = Wq.astype(bf16)
    wk_b = Wk.astype(bf16)
    wv_b = Wv.astype(bf16)
    in_maps = []
    for c in range(N_CORES):
        b, g = divmod(c, N_CORES // B)
        cs = slice(CW * g, CW * (g + 1))
        in_maps.append({
            "hsT": hsT[b],
            "wq": np.ascontiguousarray(wq_b[:, cs]),
            "wk": np.ascontiguousarray(wk_b[:, cs]),
            "wv": np.ascontiguousarray(wv_b[:, cs]),
            "bq": np.ascontiguousarray(bq[cs]).astype(np.float32),
            "mask": np.ascontiguousarray(
                attention_mask[b, 0, 0, :]).astype(np.float32),
        })
    return in_maps


def kernel(hidden_states, attention_mask, Wq, bq, Wk, bk, Wv, bv):
    from concourse.bass_utils import run_bass_kernel_spmd

    hidden_states = np.asarray(hidden_states, dtype=np.float32)
    attention_mask = np.asarray(attention_mask, dtype=np.float32)
    Wq = np.asarray(Wq, dtype=np.float32)
    Wk = np.asarray(Wk, dtype=np.float32)
    Wv = np.asarray(Wv, dtype=np.float32)
    bq = np.asarray(bq, dtype=np.float32)
    bv = np.asarray(bv, dtype=np.float32)

    in_maps = _shard_inputs(hidden_states, attention_mask, Wq, bq, Wk, Wv)
    res = run_bass_kernel_spmd(_get_nc(), in_maps, core_ids=list(range(N_CORES)))

    out = np.empty((B, S, D), dtype=np.float32)
    for c in range(N_CORES):
        b, g = divmod(c, N_CORES // B)
        out[b, :, CW * g:CW * (g + 1)] = res.results[c]["out"]
    out += bv  # exact: probs rows sum to 1
    return out


# revision 12
# speedup vs baseline: 1.1259x; 1.1259x over previous
"""BertSelfAttention on 8 Trainium2 NeuronCores (Bass/Tile, SPMD) — v2.

Problem: B=2, S=2048, D=1024, H=16 heads, head_dim=64.
Sharding: core c handles batch b = c//4 and heads [4*(c%4), 4*(c%4)+4)
(data parallel on B x tensor parallel on heads). Scores stay core-local.

v2 design (vs v1 baseline at 262us):
  - ACT(exp) is the hard floor: 16.8M probs/core at 1 elem/lane/cycle
    @1.2GHz + ~280cyc/call => ~141us for 128 [128,1024] calls. The whole
    kernel is scheduled so the exp stream NEVER stalls:
      * pT staged in SBUF (deep ring) decouples ctx matmuls from exp.
      * ~10 kc of scores+exp prefetched while the V projection runs.
      * scores matmuls are pair-tiled: heads 2j/2j+1 run CONCURRENTLY in
        the PE via tile_position row groups (0,0)/(64,0) with K=64 (kT
        stored compact, no zero padding) -> PE well under ACT per kc.
  - bf16 everywhere on inputs/activations (halves DMA + SBUF; matmul
    rate on PE is column-driven, so bf16 == fp32r there). PSUM fp32.
  - out-DMAs alternate sync/gpsimd queues to kill the tail.

Math notes (exact transformations vs the reference):
  - bk dropped: scores[i,j] += q_i . bk is constant in j -> softmax invariant.
  - bv added host-side: softmax rows sum to 1 -> probs @ (1 x bv) = bv.
  - no max-subtraction: scores ~ N(0,1), exp range is tiny for fp32.
  - additive mask folded multiplicatively: exp(s+m) = exp(s)*exp(m), with
    exp(mask) baked into V columns and a 65th "ones" column per head whose
    ctx row is the softmax denominator.
"""

import numpy as np
from contextlib import ExitStack

B, S, D, H = 2, 2048, 1024, 16
HD = 64
N_CORES = 8
HPC = 4            # heads per core
CW = HPC * HD      # 256 output cols per core
KI = D // 128      # 8 contraction chunks
NSC = S // 128     # 16 k-chunks of 128


_NC_CACHE = []


def _build_nc():
    import concourse.bacc as bacc
    import concourse.mybir as mybir
    import concourse.tile as tile
    from concourse import masks

    F32 = mybir.dt.float32
    F32R = mybir.dt.float32r
    BF16 = mybir.dt.bfloat16
    AF = mybir.ActivationFunctionType

    nc = bacc.Bacc("TRN2", target_bir_lowering=False, debug=False)

    hsT_d = nc.dram_tensor("hsT", [D, S], BF16, kind="ExternalInput")
    wq_d = nc.dram_tensor("wq", [D, CW], BF16, kind="ExternalInput")
    wk_d = nc.dram_tensor("wk", [D, CW], BF16, kind="ExternalInput")
    wv_d = nc.dram_tensor("wv", [D, CW], BF16, kind="ExternalInput")
    bq_d = nc.dram_tensor("bq", [CW], F32, kind="ExternalInput")
    mask_d = nc.dram_tensor("mask", [S], F32, kind="ExternalInput")
    out_d = nc.dram_tensor("out", [S, CW], F32, kind="ExternalOutput")

    hsT_r = hsT_d.rearrange("(ki p) s -> p ki s", p=128)
    wq_r = wq_d.rearrange("(ki p) n -> p ki n", p=128)
    wk_r = wk_d.rearrange("(ki p) n -> p ki n", p=128)
    wv_r = wv_d.rearrange("(ki p) n -> p ki n", p=128)
    bq_r = bq_d.rearrange("(j p) -> p j", p=128)
    mask_r = mask_d.rearrange("(sc p) -> p sc", p=128)

    with tile.TileContext(nc) as tc, ExitStack() as ctx:
        const = ctx.enter_context(tc.tile_pool(name="const", bufs=1))
        load = ctx.enter_context(tc.tile_pool(name="load", bufs=1))
        acts = ctx.enter_context(tc.tile_pool(name="acts", bufs=1))
        ptp = ctx.enter_context(tc.tile_pool(name="ptp", bufs=8))
        ctsp = ctx.enter_context(tc.tile_pool(name="ctsp", bufs=4))
        outp = ctx.enter_context(tc.tile_pool(name="outp", bufs=8))
        # PSUM (8 banks): sp 2x[128,1024] = 4 banks; acc 2x[65,1024] = 4
        # banks (transposes reuse the acc ring between qh phases).
        sp = ctx.enter_context(tc.tile_pool(name="sp", bufs=2, space="PSUM"))
        accp = ctx.enter_context(tc.tile_pool(name="accp", bufs=2, space="PSUM"))

        # ---- constants ----
        ident = const.tile([128, 128], BF16)
        masks.make_identity(nc, ident[:])
        bq_t = const.tile([128, 2], F32)
        nc.sync.dma_start(bq_t[:], bq_r)
        maskr = const.tile([128, 16], F32)
        nc.sync.dma_start(maskr[:], mask_r)
        emt = const.tile([128, 16], F32)
        nc.scalar.activation(emt[:], maskr[:], AF.Exp)

        # ---- loads ----
        hsT_t = load.tile([128, KI, S], BF16)
        wq_t = load.tile([128, KI, CW], BF16)
        wk_t = load.tile([128, KI, CW], BF16)
        wv_t = load.tile([128, KI, CW], BF16)
        for ki in range(KI):
            nc.gpsimd.dma_start(wq_t[:, ki, :], wq_r[:, ki, :])
        for ki in range(KI):
            nc.gpsimd.dma_start(wk_t[:, ki, :], wk_r[:, ki, :])
        # hsT per (n, ki) so qk(j0, n) can start as chunks land
        for n in range(4):
            for ki in range(KI):
                nc.sync.dma_start(hsT_t[:, ki, 512 * n:512 * (n + 1)],
                                  hsT_r[:, ki, 512 * n:512 * (n + 1)])
        for ki in range(KI):
            nc.gpsimd.dma_start(wv_t[:, ki, :], wv_r[:, ki, :])

        # ---- persistent activations ----
        qT_t = acts.tile([128, 2, S], BF16)
        kz = acts.tile([128, 2, S], BF16)
        v_ext = acts.tile([128, NSC, HPC * (HD + 1)], F32R)

        def emit_qk_n(j, n):
                ps = sp.tile([128, 1024], F32, tag="sp", name=f"qk{j}_{n}")
                for ki in range(KI):
                    nc.tensor.matmul(
                        ps[:, 0:512], wq_t[:, ki, 128 * j:128 * (j + 1)],
                        hsT_t[:, ki, 512 * n:512 * (n + 1)],
                        start=(ki == 0), stop=(ki == KI - 1))
                for ki in range(KI):
                    nc.tensor.matmul(
                        ps[:, 512:1024], wk_t[:, ki, 128 * j:128 * (j + 1)],
                        hsT_t[:, ki, 512 * n:512 * (n + 1)],
                        start=(ki == 0), stop=(ki == KI - 1))
                nc.vector.tensor_scalar_add(qT_t[:, j, 512 * n:512 * (n + 1)],
                                            ps[:, 0:512], bq_t[:, j:j + 1])
                nc.vector.tensor_copy(kz[:, j, 512 * n:512 * (n + 1)],
                                      ps[:, 512:1024])

        def emit_v_chunk(sc):
                ps = sp.tile([128, 1024], F32, tag="sp", name=f"v{sc}")
                for ki in range(KI):
                    nc.tensor.matmul(
                        ps[:, 0:CW], hsT_t[:, ki, 128 * sc:128 * (sc + 1)],
                        wv_t[:, ki, :], start=(ki == 0), stop=(ki == KI - 1))
                for h in range(HPC):
                    nc.vector.tensor_scalar_mul(
                        v_ext[:, sc, 65 * h:65 * h + 64],
                        ps[:, 64 * h:64 * (h + 1)], emt[:, sc:sc + 1])
                    nc.vector.tensor_copy(
                        v_ext[:, sc, 65 * h + 64:65 * h + 65], emt[:, sc:sc + 1])

        def emit_scores_exp(j, qh, kc):
            """Pair-tiled scores for heads (2j, 2j+1) + exp -> (pA, pB)."""
            spA = sp.tile([128, 1024], F32, tag="sp", name=f"sA{j}_{qh}_{kc}")
            spB = sp.tile([128, 1024], F32, tag="sp", name=f"sB{j}_{qh}_{kc}")
            q0 = 1024 * qh
            for half in range(2):
                qs = slice(q0 + 512 * half, q0 + 512 * (half + 1))
                os_ = slice(512 * half, 512 * (half + 1))
                nc.tensor.matmul(
                    spA[:, os_], kz[0:64, j, 128 * kc:128 * (kc + 1)],
                    qT_t[0:64, j, qs], start=True, stop=True,
                    tile_position=(0, 0))
                nc.tensor.matmul(
                    spB[:, os_], kz[64:128, j, 128 * kc:128 * (kc + 1)],
                    qT_t[64:128, j, qs], start=True, stop=True,
                    tile_position=(64, 0))
            pA = ptp.tile([128, 1024], F32R, tag="pt", name=f"pA{j}_{qh}_{kc}")
            pB = ptp.tile([128, 1024], F32R, tag="pt", name=f"pB{j}_{qh}_{kc}")
            nc.scalar.activation(pA[:], spA[:], AF.Exp, scale=0.125)
            nc.scalar.activation(pB[:], spB[:], AF.Exp, scale=0.125)
            return pA, pB

        def emit_attention(j, qh, aux=None):
            accA = accp.tile([65, 1024], F32, tag="acc", name=f"aA{j}_{qh}")
            accB = accp.tile([65, 1024], F32, tag="acc", name=f"aB{j}_{qh}")
            for kc in range(NSC):
                pA, pB = emit_scores_exp(j, qh, kc)
                if aux is not None and kc in aux:
                    aux[kc]()
                for acc, p, hh in ((accA, pA, 0), (accB, pB, 1)):
                    h = 2 * j + hh
                    for half in range(2):
                        os_ = slice(512 * half, 512 * (half + 1))
                        nc.tensor.matmul(
                            acc[0:65, os_], v_ext[:, kc, 65 * h:65 * (h + 1)],
                            p[:, os_], start=(kc == 0), stop=(kc == NSC - 1))
            # epilogue: transpose, normalize, store
            for hh, acc in ((0, accA), (1, accB)):
                h = 2 * j + hh
                cts = ctsp.tile([65, 1024], BF16, tag="cts")
                nc.vector.tensor_copy(cts[:], acc[0:65, :])
                for sq in range(8):
                    tp = accp.tile([128, 65], BF16, tag="acc",
                                   name=f"tp{j}_{qh}_{hh}_{sq}")
                    nc.tensor.transpose(tp[:, 0:65],
                                        cts[0:65, 128 * sq:128 * (sq + 1)],
                                        ident[0:65, 0:65])
                    rec = outp.tile([128, 1], F32, tag="rec")
                    nc.vector.reciprocal(rec[:], tp[:, 64:65])
                    ot = outp.tile([128, HD], F32, tag="ot")
                    nc.vector.tensor_scalar_mul(ot[:], tp[:, 0:HD], rec[:, 0:1])
                    row = 1024 * qh + 128 * sq
                    eng = nc.sync if sq % 2 == 0 else nc.gpsimd
                    eng.dma_start(
                        out_d[row:row + 128, HD * h:HD * (h + 1)], ot[:])

        # ---- schedule ----
        # Every window is lag-0 self-interleaved (scores -> exp -> ctx per
        # kc); the exp stream starts right after qk(j0). The V projection
        # threads one sc-chunk per kc through window 0 (ctx(W0,kc) needs
        # v_ext[:,kc] exactly then), and qk(j1) threads through window 1 in
        # four chunks. Both ride the same sp-ring as the score tiles, which
        # costs a small bounded slip per chunk instead of a ring stall.
        for n in range(4):
            emit_qk_n(0, n)
        emit_attention(0, 0, aux={kc: (lambda sc=kc: emit_v_chunk(sc))
                                  for kc in range(NSC)})
        emit_attention(0, 1, aux={kc: (lambda n=kc // 4: emit_qk_n(1, n))
                                  for kc in (1, 5, 9, 13)})
        emit_attention(1, 0)
        emit_attention(1, 1)

    nc.finalize()
    return nc


def _get_nc():
    if not _NC_CACHE:
        _NC_CACHE.append(_build_nc())
    return _NC_CACHE[0]


def _shard_inputs(hidden_states, attention_mask, Wq, bq, Wk, Wv):
    import ml_dtypes
    bf16 = ml_dtypes.bfloat16
    hsT = [np.ascontiguousarray(hidden_states[b].T).astype(bf16)
           for b in range(B)]
    wq_b = Wq.astype(bf16)
    wk_b = Wk.astype(bf16)
    wv_b = Wv.astype(bf16)
    in_maps = []
    for c in range(N_CORES):
        b, g = divmod(c, N_CORES // B)
        cs = slice(CW * g, CW * (g + 1))
        in_maps.append({
            "hsT": hsT[b],
            "wq": np.ascontiguousarray(wq_b[:, cs]),
            "wk": np.ascontiguousarray(wk_b[:, cs]),
            "wv": np.ascontiguousarray(wv_b[:, cs]),
            "bq": np.ascontiguousarray(bq[cs]).astype(np.float32),
            "mask": np.ascontiguousarray(
                attention_mask[b, 0, 0, :]).astype(np.float32),
        })
    return in_maps


def kernel(hidden_states, attention_mask, Wq, bq, Wk, bk, Wv, bv):
    from concourse.bass_utils import run_bass_kernel_spmd

    hidden_states = np.asarray(hidden_states, dtype=np.float32)
    attention_mask = np.asarray(attention_mask, dtype=np.float32)
    Wq = np.asarray(Wq, dtype=np.float32)
    Wk = np.asarray(Wk, dtype=np.float32)
    Wv = np.asarray(Wv, dtype=np.float32)
    bq = np.asarray(bq, dtype=np.float32)
    bv = np.asarray(bv, dtype=np.float32)

    in_maps = _shard_inputs(hidden_states, attention_mask, Wq, bq, Wk, Wv)
    res = run_bass_kernel_spmd(_get_nc(), in_maps, core_ids=list(range(N_CORES)))

    out = np.empty((B, S, D), dtype=np.float32)
    for c in range(N_CORES):
        b, g = divmod(c, N_CORES // B)
        out[b, :, CW * g:CW * (g + 1)] = res.results[c]["out"]
    out += bv  # exact: probs rows sum to 1
    return out


# revision 15
# speedup vs baseline: 1.2631x; 1.1218x over previous
"""BertSelfAttention on 8 Trainium2 NeuronCores (Bass/Tile, SPMD) — v4.

Problem: B=2, S=2048, D=1024, H=16 heads, head_dim=64.
Sharding: core c handles batch b = c//4 and heads [4*(c%4), 4*(c%4)+4)
(data parallel on B x tensor parallel on heads). Scores stay core-local.

Design: ACT(exp) is the hard floor — 16.8M probs/core at 1 elem/lane/cyc
@1.2GHz + ~280cyc/call = ~140us over 128 [128,1024] calls. Everything is
scheduled so the exp stream never waits:
  - Each q-window (j-pair x q-half) runs scores -> exp -> ctx with ctx
    LAGGING ONE kc, so the PE always has ready work while exps drain.
  - Scores are pair-tiled: heads 2j/2j+1 run concurrently on the PE via
    tile_position row groups (0,0)/(64,0) with K=64 (kT compact, no
    zero-padding).
  - The V projection (window 0) and qk(j1) (window 1) are threaded one
    small chunk per kc through the strict-FIFO score-tile ring; chunk
    lifetime is kept under one exp period so the ring never stalls.
  - pT/v_ext are f32r (bf16 ACT/DVE *outputs* measured slower); inputs
    and q/k activations are bf16 (half DMA + SBUF, same PE rate).
  - Epilogues (transpose+normalize+store) use one [128,8,65] PSUM slab
    per head and trail the compute by one window; out-DMAs alternate
    sync/gpsimd queues.

Math notes (exact transformations vs the reference):
  - bk dropped: scores[i,j] += q_i . bk is constant in j -> softmax invariant.
  - bv added host-side: softmax rows sum to 1 -> probs @ (1 x bv) = bv.
  - no max-subtraction: scores ~ N(0,1), exp range is tiny for fp32.
  - additive mask folded multiplicatively: exp(s+m) = exp(s)*exp(m), with
    exp(mask) baked into V columns and a 65th "ones" column per head whose
    ctx row is the softmax denominator.
"""

import numpy as np
from contextlib import ExitStack

B, S, D, H = 2, 2048, 1024, 16
HD = 64
N_CORES = 8
HPC = 4            # heads per core
CW = HPC * HD      # 256 output cols per core
KI = D // 128      # 8 contraction chunks
NSC = S // 128     # 16 k-chunks of 128

_NC_CACHE = []


def _build_nc():
    import concourse.bacc as bacc
    import concourse.mybir as mybir
    import concourse.tile as tile
    from concourse import masks

    F32 = mybir.dt.float32
    F32R = mybir.dt.float32r
    BF16 = mybir.dt.bfloat16
    AF = mybir.ActivationFunctionType

    nc = bacc.Bacc("TRN2", target_bir_lowering=False, debug=False)

    hsT_d = nc.dram_tensor("hsT", [D, S], BF16, kind="ExternalInput")
    wq_d = nc.dram_tensor("wq", [D, CW], BF16, kind="ExternalInput")
    wk_d = nc.dram_tensor("wk", [D, CW], BF16, kind="ExternalInput")
    wv_d = nc.dram_tensor("wv", [D, CW], BF16, kind="ExternalInput")
    bq_d = nc.dram_tensor("bq", [CW], F32, kind="ExternalInput")
    mask_d = nc.dram_tensor("mask", [S], F32, kind="ExternalInput")
    out_d = nc.dram_tensor("out", [S, CW], F32, kind="ExternalOutput")

    hsT_r = hsT_d.rearrange("(ki p) s -> p ki s", p=128)
    wq_r = wq_d.rearrange("(ki p) n -> p ki n", p=128)
    wk_r = wk_d.rearrange("(ki p) n -> p ki n", p=128)
    wv_r = wv_d.rearrange("(ki p) n -> p ki n", p=128)
    bq_r = bq_d.rearrange("(j p) -> p j", p=128)
    mask_r = mask_d.rearrange("(sc p) -> p sc", p=128)

    with tile.TileContext(nc) as tc, ExitStack() as ctx:
        const = ctx.enter_context(tc.tile_pool(name="const", bufs=1))
        load = ctx.enter_context(tc.tile_pool(name="load", bufs=1))
        acts = ctx.enter_context(tc.tile_pool(name="acts", bufs=1))
        ptp = ctx.enter_context(tc.tile_pool(name="ptp", bufs=8))
        ctsp = ctx.enter_context(tc.tile_pool(name="ctsp", bufs=4))
        outp = ctx.enter_context(tc.tile_pool(name="outp", bufs=10))
        # PSUM (8 banks): sp ring 2x[128,1024] = 4 banks; accp ring
        # 2x[65,1024] = 4 banks (epilogue tp-slabs ride the accp ring).
        sp = ctx.enter_context(tc.tile_pool(name="sp", bufs=2, space="PSUM"))
        accp = ctx.enter_context(tc.tile_pool(name="accp", bufs=2, space="PSUM"))

        # ---- constants ----
        ident = const.tile([128, 128], BF16)
        masks.make_identity(nc, ident[:])
        bq_t = const.tile([128, 2], F32)
        nc.sync.dma_start(bq_t[:], bq_r)
        maskr = const.tile([128, 16], F32)
        nc.sync.dma_start(maskr[:], mask_r)
        emt = const.tile([128, 16], F32)
        nc.scalar.activation(emt[:], maskr[:], AF.Exp)

        # ---- loads ----
        hsT_t = load.tile([128, KI, S], BF16)
        wq_t = load.tile([128, KI, CW], BF16)
        wk_t = load.tile([128, KI, CW], BF16)
        wv_t = load.tile([128, KI, CW], BF16)
        for ki in range(KI):
            nc.gpsimd.dma_start(wq_t[:, ki, :], wq_r[:, ki, :])
        for ki in range(KI):
            nc.gpsimd.dma_start(wk_t[:, ki, :], wk_r[:, ki, :])
        # hsT per (n, ki) so qk(j0, n) can start as chunks land
        for n in range(4):
            for ki in range(KI):
                nc.sync.dma_start(hsT_t[:, ki, 512 * n:512 * (n + 1)],
                                  hsT_r[:, ki, 512 * n:512 * (n + 1)])
        for ki in range(KI):
            nc.gpsimd.dma_start(wv_t[:, ki, :], wv_r[:, ki, :])

        # ---- persistent activations ----
        qT_t = acts.tile([128, 2, S], BF16)
        kz = acts.tile([128, 2, S], BF16)
        v_ext = acts.tile([128, NSC, HPC * (HD + 1)], F32R)
        # ones-columns (exp(mask) per k-chunk) written once up front
        for h in range(HPC):
            nc.vector.tensor_copy(v_ext[:, :, 65 * h + 64], emt[:, :])

        def emit_qk_n(j, n):
            ps = sp.tile([128, 1024], F32, tag="sp", name=f"qk{j}_{n}")
            for ki in range(KI):
                nc.tensor.matmul(
                    ps[:, 0:512], wq_t[:, ki, 128 * j:128 * (j + 1)],
                    hsT_t[:, ki, 512 * n:512 * (n + 1)],
                    start=(ki == 0), stop=(ki == KI - 1))
            for ki in range(KI):
                nc.tensor.matmul(
                    ps[:, 512:1024], wk_t[:, ki, 128 * j:128 * (j + 1)],
                    hsT_t[:, ki, 512 * n:512 * (n + 1)],
                    start=(ki == 0), stop=(ki == KI - 1))
            nc.vector.tensor_scalar_add(qT_t[:, j, 512 * n:512 * (n + 1)],
                                        ps[:, 0:512], bq_t[:, j:j + 1])
            nc.vector.tensor_copy(kz[:, j, 512 * n:512 * (n + 1)],
                                  ps[:, 512:1024])

        def emit_qk_quarter(j, i):
            """1/16th of qk(j): 256 cols of q or k for n-chunk i//4."""
            n, half, sub = i // 4, (i // 2) % 2, i % 2
            cs = slice(512 * n + 256 * sub, 512 * n + 256 * (sub + 1))
            ps = sp.tile([128, 256], F32, tag="sp", name=f"qk{j}_q{i}")
            w_t = wq_t if half == 0 else wk_t
            for ki in range(KI):
                nc.tensor.matmul(
                    ps[:], w_t[:, ki, 128 * j:128 * (j + 1)],
                    hsT_t[:, ki, cs], start=(ki == 0), stop=(ki == KI - 1))
            if half == 0:
                nc.vector.tensor_scalar_add(qT_t[:, j, cs], ps[:],
                                            bq_t[:, j:j + 1])
            else:
                nc.vector.tensor_copy(kz[:, j, cs], ps[:])

        def emit_v_chunk(sc):
            ps = sp.tile([128, 256], F32, tag="sp", name=f"v{sc}")
            for ki in range(KI):
                nc.tensor.matmul(
                    ps[:], hsT_t[:, ki, 128 * sc:128 * (sc + 1)],
                    wv_t[:, ki, :], start=(ki == 0), stop=(ki == KI - 1))
            for h in range(HPC):
                nc.vector.tensor_scalar_mul(
                    v_ext[:, sc, 65 * h:65 * h + 64],
                    ps[:, 64 * h:64 * (h + 1)], emt[:, sc:sc + 1])

        def emit_scores_exp(j, qh, kc):
            """Pair-tiled scores for heads (2j, 2j+1) + exp -> (pA, pB)."""
            spA = sp.tile([128, 1024], F32, tag="sp", name=f"sA{j}_{qh}_{kc}")
            spB = sp.tile([128, 1024], F32, tag="sp", name=f"sB{j}_{qh}_{kc}")
            q0 = 1024 * qh
            for half in range(2):
                qs = slice(q0 + 512 * half, q0 + 512 * (half + 1))
                os_ = slice(512 * half, 512 * (half + 1))
                nc.tensor.matmul(
                    spA[:, os_], kz[0:64, j, 128 * kc:128 * (kc + 1)],
                    qT_t[0:64, j, qs], start=True, stop=True,
                    tile_position=(0, 0))
                nc.tensor.matmul(
                    spB[:, os_], kz[64:128, j, 128 * kc:128 * (kc + 1)],
                    qT_t[64:128, j, qs], start=True, stop=True,
                    tile_position=(64, 0))
            pA = ptp.tile([128, 1024], F32R, tag="pt", name=f"pA{j}_{qh}_{kc}")
            pB = ptp.tile([128, 1024], F32R, tag="pt", name=f"pB{j}_{qh}_{kc}")
            nc.scalar.activation(pA[:], spA[:], AF.Exp, scale=0.125)
            nc.scalar.activation(pB[:], spB[:], AF.Exp, scale=0.125)
            return pA, pB

        def emit_ctx(j, accA, accB, kc, pAB):
            pA, pB = pAB
            for acc, p, hh in ((accA, pA, 0), (accB, pB, 1)):
                h = 2 * j + hh
                for half in range(2):
                    os_ = slice(512 * half, 512 * (half + 1))
                    nc.tensor.matmul(
                        acc[0:65, os_], v_ext[:, kc, 65 * h:65 * (h + 1)],
                        p[:, os_], start=(kc == 0), stop=(kc == NSC - 1))

        def emit_attention(j, qh, aux=None):
            """One q-window: scores+exp per kc, ctx lagging one kc."""
            accA = accp.tile([65, 1024], F32, tag="acc", name=f"aA{j}_{qh}")
            accB = accp.tile([65, 1024], F32, tag="acc", name=f"aB{j}_{qh}")
            prev = None
            for kc in range(NSC):
                cur = emit_scores_exp(j, qh, kc)
                if aux is not None and kc in aux:
                    aux[kc]()
                if prev is not None:
                    emit_ctx(j, accA, accB, kc - 1, prev)
                prev = cur
            emit_ctx(j, accA, accB, NSC - 1, prev)
            return accA, accB

        def emit_epilogue(j, qh, accA, accB):
            """Transpose ctx^T -> [q, hd], divide by denominator, store."""
            for hh, acc in ((0, accA), (1, accB)):
                h = 2 * j + hh
                cts = ctsp.tile([65, 1024], BF16, tag="cts")
                nc.vector.tensor_copy(cts[:], acc[0:65, :])
                tps = accp.tile([128, 8, 66], BF16, tag="acc",
                                name=f"tp{j}_{qh}_{hh}")
                for sq in range(8):
                    nc.tensor.transpose(tps[:, sq, 0:65],
                                        cts[0:65, 128 * sq:128 * (sq + 1)],
                                        ident[0:65, 0:65])
                rec = outp.tile([128, 8], F32, tag="rec")
                nc.vector.reciprocal(rec[:], tps[:, :, 64])
                for sq in range(8):
                    ot = outp.tile([128, HD], F32, tag="ot")
                    nc.vector.tensor_scalar_mul(ot[:], tps[:, sq, 0:HD],
                                                rec[:, sq:sq + 1])
                    row = 1024 * qh + 128 * sq
                    eng = nc.sync if sq % 2 == 0 else nc.gpsimd
                    eng.dma_start(
                        out_d[row:row + 128, HD * h:HD * (h + 1)], ot[:])

        # ---- schedule ----
        for n in range(4):
            emit_qk_n(0, n)
        a00 = emit_attention(
            0, 0, aux={kc: (lambda sc=kc: emit_v_chunk(sc))
                       for kc in range(NSC)})
        emit_epilogue(0, 0, *a00)
        a01 = emit_attention(
            0, 1, aux={kc: (lambda i=kc: emit_qk_quarter(1, i))
                       for kc in range(NSC)})
        emit_epilogue(0, 1, *a01)
        a10 = emit_attention(1, 0)
        emit_epilogue(1, 0, *a10)
        a11 = emit_attention(1, 1)
        emit_epilogue(1, 1, *a11)

    nc.finalize()
    return nc


def _get_nc():
    if not _NC_CACHE:
        _NC_CACHE.append(_build_nc())
    return _NC_CACHE[0]


def _shard_inputs(hidden_states, attention_mask, Wq, bq, Wk, Wv):
    import ml_dtypes
    bf16 = ml_dtypes.bfloat16
    hsT = [np.ascontiguousarray(hidden_states[b].T).astype(bf16)
           for b in range(B)]
    wq_b = Wq.astype(bf16)
    wk_b = Wk.astype(bf16)
    wv_b = Wv.astype(bf16)
    in_maps = []
    for c in range(N_CORES):
        b, g = divmod(c, N_CORES // B)
        cs = slice(CW * g, CW * (g + 1))
        in_maps.append({
            "hsT": hsT[b],
            "wq": np.ascontiguousarray(wq_b[:, cs]),
            "wk": np.ascontiguousarray(wk_b[:, cs]),
            "wv": np.ascontiguousarray(wv_b[:, cs]),
            "bq": np.ascontiguousarray(bq[cs]).astype(np.float32),
            "mask": np.ascontiguousarray(
                attention_mask[b, 0, 0, :]).astype(np.float32),
        })
    return in_maps


def kernel(hidden_states, attention_mask, Wq, bq, Wk, bk, Wv, bv):
    from concourse.bass_utils import run_bass_kernel_spmd

    hidden_states = np.asarray(hidden_states, dtype=np.float32)
    attention_mask = np.asarray(attention_mask, dtype=np.float32)
    Wq = np.asarray(Wq, dtype=np.float32)
    Wk = np.asarray(Wk, dtype=np.float32)
    Wv = np.asarray(Wv, dtype=np.float32)
    bq = np.asarray(bq, dtype=np.float32)
    bv = np.asarray(bv, dtype=np.float32)

    in_maps = _shard_inputs(hidden_states, attention_mask, Wq, bq, Wk, Wv)
    res = run_bass_kernel_spmd(_get_nc(), in_maps, core_ids=list(range(N_CORES)))

    out = np.empty((B, S, D), dtype=np.float32)
    for c in range(N_CORES):
        b, g = divmod(c, N_CORES // B)
        out[b, :, CW * g:CW * (g + 1)] = res.results[c]["out"]
    out += bv  # exact: probs rows sum to 1
    return out
